# revision 13
# baseline (speedup 1.0000x reference)
"""Trainium2 Bass kernel for the autoregressive GRU decoder.

Reference computation (eval-mode Decoder):
  x0 = x[:, 30, :]                # only element of x ever used
  h0 = h[0]
  for t in 0..29:
      h = GRUCell(x_t, h)         # PyTorch gate layout [r, z, n]
      y_t = h @ W_out.T + b_out
      x_{t+1} = y_t               # linear feedback -> fold into weights
  out = stack(y_t)                # [B, 30, 32]

Because the feedback x_{t+1} = W_out @ h_t + b_out is linear, for t >= 1:
  gi_t = W_ih @ x_t + b_ih = (W_ih @ W_out) @ h_{t-1} + (W_ih @ b_out + b_ih)
so every step t >= 1 is a pure H->H recurrence.  Weights are folded on the
host; r/z gates use a single combined matrix (W_hh + W_ih_eff).

Layout on device (per core, batch shard Bc = 2048):
  state h^T kept transposed: [H=128 partitions, Bc free].  All gate matmuls
  contract over H with gate weights as the stationary operand; per-partition
  gate biases ride along for free in ACT bias / scalar_tensor_tensor slots.
  y_t is computed "direct" per 128-batch tile with h'^T as the stationary
  operand, giving [batch, 32] PSUM tiles that stage into an SBUF output
  buffer with contiguous 3840B rows for the final DMA.

Sharding: pure data parallel over batch, 8 cores x 2048, no collectives.
"""

import os

import numpy as np

B, T, I, H, SEQLEN = 16384, 60, 32, 128, 30
STEPS = T - SEQLEN  # 30
NCORES = 8
BC = B // NCORES  # 2048 batch rows per core
NB = BC // 128  # 16 batch tiles of 128
CH = 512  # matmul free-dim chunk (one PSUM bank)
NCH = BC // CH  # 4 chunks per step

# dtype knobs:
#   K_MM_DT: f32 | f32r | bf16   (matmul operand dtype)
#   K_EW_DT: f32 | bf16          (h/n/z/w/e elementwise storage dtype)
MM_DT = os.environ.get("K_MM_DT", "f32r")
EW_DT = os.environ.get("K_EW_DT", "f32")

LAST_RESULT = None  # BassKernelResults of the most recent run (for test.py)

_CACHE = {}


def _build(mm_dt_s, ew_dt_s):
    from contextlib import ExitStack

    import concourse.bacc as bacc
    import concourse.bass as bass  # noqa: F401
    import concourse.mybir as mybir
    import concourse.tile as tile

    f32 = mybir.dt.float32
    bf16 = mybir.dt.bfloat16
    f32r = mybir.dt.float32r
    Alu = mybir.AluOpType
    Act = mybir.ActivationFunctionType

    ewdt = {"f32": f32, "bf16": bf16}[ew_dt_s]
    # matmul operand dtype.  f32r must be a *declared* dtype end-to-end
    # (DRAM tensor -> SBUF tile -> engine out dtype); the BIR verifier
    # rejects fp32-producer -> f32r-matmul chains.
    if ew_dt_s == "bf16":
        mmdt = bf16  # state is 2-byte; matmuls must read it as bf16
    else:
        mmdt = {"f32": f32, "f32r": f32r}[mm_dt_s]
    # dtype for DRAM/SBUF storage of matmul-feeding tensors
    mstore = mmdt if mmdt == f32r else (bf16 if ewdt == bf16 else f32)
    # state tiles feed matmuls directly, so they take the matmul dtype
    # when in f32r mode (bit-identical to fp32 elsewhere)
    hdt = f32r if mmdt == f32r else ewdt

    # Bacc (not plain Bass): its compile() pipeline splits multi-sem waits
    # into event-semaphore chains — raw Bass emits >1-wait instructions
    # that walrus codegen rejects ("Too many sync wait commands").
    nc = bacc.Bacc()

    # matmul-feeding inputs are declared in the f32r/f32 storage dtype
    wdt = f32r if mmdt == f32r else f32
    dx = nc.dram_tensor("x0t", [I, BC], wdt, kind="ExternalInput")
    dh = nc.dram_tensor("h0t", [H, BC], wdt, kind="ExternalInput")
    dwa = nc.dram_tensor("wa", [H, 4 * H], wdt, kind="ExternalInput")
    dwa0 = nc.dram_tensor("wa0", [H, 2 * H], wdt, kind="ExternalInput")
    dw0 = nc.dram_tensor("w0", [I, 3 * H], wdt, kind="ExternalInput")
    dwout = nc.dram_tensor("woutt", [H, I], wdt, kind="ExternalInput")
    dbias = nc.dram_tensor("bias", [H, 8], f32, kind="ExternalInput")
    dbout = nc.dram_tensor("bout", [128, I], f32, kind="ExternalInput")
    dout = nc.dram_tensor("out", [BC, STEPS, I], f32, kind="ExternalOutput")

    with ExitStack() as ctx:
        tc = ctx.enter_context(tile.TileContext(nc))
        const = ctx.enter_context(tc.tile_pool(name="const", bufs=1))
        state = ctx.enter_context(tc.tile_pool(name="state", bufs=2))
        work = ctx.enter_context(tc.tile_pool(name="work", bufs=3))
        ypool = ctx.enter_context(tc.tile_pool(name="ystage", bufs=1))
        psum = ctx.enter_context(tc.tile_pool(name="psum", bufs=1, space="PSUM"))

        # ---- constants ----
        def load_const(dram, shape, name, dt_store):
            t = const.tile(shape, dram.dtype, tag=name)
            nc.sync.dma_start(out=t[:], in_=dram[:, :])
            if dt_store != dram.dtype:
                tt = const.tile(shape, dt_store, tag=name + "_c")
                nc.vector.tensor_copy(tt[:], t[:])
                return tt
            return t

        swa = load_const(dwa, [H, 4 * H], "wa", mmdt)
        swa0 = load_const(dwa0, [H, 2 * H], "wa0", mmdt)
        sw0 = load_const(dw0, [I, 3 * H], "w0", mmdt)
        swout = load_const(dwout, [H, I], "wout", mmdt)
        sbias = load_const(dbias, [H, 8], "bias", f32)  # biases always fp32
        sbout = load_const(dbout, [128, I], "bout", f32)
        sx0 = load_const(dx, [I, BC], "x0", mmdt)

        b_r = sbias[:, 0:1]
        b_z = sbias[:, 1:2]
        b_hn = sbias[:, 2:3]
        b_in = sbias[:, 3:4]
        b0_r = sbias[:, 4:5]
        b0_z = sbias[:, 5:6]
        b0_in = sbias[:, 6:7]

        A_r = swa[:, 0 * H : 1 * H]
        A_z = swa[:, 1 * H : 2 * H]
        A_hn = swa[:, 2 * H : 3 * H]
        A_in = swa[:, 3 * H : 4 * H]
        A0_r = swa0[:, 0 * H : 1 * H]
        A0_z = swa0[:, 1 * H : 2 * H]
        W0_r = sw0[:, 0 * H : 1 * H]
        W0_z = sw0[:, 1 * H : 2 * H]
        W0_n = sw0[:, 2 * H : 3 * H]
        WoutT = swout[:, :]

        # initial state (cast if state dtype differs from DRAM dtype)
        h_prev = state.tile([H, BC], hdt, tag="h")
        if hdt == dh.dtype:
            nc.sync.dma_start(out=h_prev[:], in_=dh[:, :])
        else:
            htmp = const.tile([H, BC], dh.dtype, tag="h0tmp")
            nc.sync.dma_start(out=htmp[:], in_=dh[:, :])
            nc.vector.tensor_copy(h_prev[:], htmp[:])

        # collapse the wait fan-in of the first matmuls: without this,
        # step-0 instructions wait on every const-load DMA individually and
        # walrus codegen overflows the per-instruction sync-wait slots.
        pass  # barrier removed

        # output staging: [128, btile, step, 32]
        Y = ypool.tile([128, NB, STEPS, I], f32, tag="Y")

        for t in range(STEPS):
            h_new = state.tile([H, BC], hdt, tag="h")
            py = psum.tile([128, NB, I], f32, tag="py", bufs=2)
            first = t == 0
            for c in range(NCH):
                cs = slice(c * CH, (c + 1) * CH)
                rhs = h_prev[:, cs]

                grz = psum.tile([128, 2 * CH], f32, tag="grz", bufs=2)
                ghn = psum.tile([128, 2 * CH], f32, tag="ghn", bufs=1)

                if first:
                    x_rhs = sx0[:, cs]
                    nc.tensor.matmul(grz[:, 0:CH], A0_r, rhs, start=True, stop=False)
                    nc.tensor.matmul(grz[:, 0:CH], W0_r, x_rhs, start=False, stop=True)
                    nc.tensor.matmul(grz[:, CH:], A0_z, rhs, start=True, stop=False)
                    nc.tensor.matmul(grz[:, CH:], W0_z, x_rhs, start=False, stop=True)
                    nc.tensor.matmul(ghn[:, 0:CH], A_hn, rhs)
                    nc.tensor.matmul(ghn[:, CH:], W0_n, x_rhs)
                    cb_r, cb_z, cb_in = b0_r, b0_z, b0_in
                else:
                    nc.tensor.matmul(grz[:, 0:CH], A_r, rhs)
                    nc.tensor.matmul(grz[:, CH:], A_z, rhs)
                    nc.tensor.matmul(ghn[:, 0:CH], A_hn, rhs)
                    nc.tensor.matmul(ghn[:, CH:], A_in, rhs)
                    cb_r, cb_z, cb_in = b_r, b_z, b_in

                r_sb = work.tile([128, CH], ewdt, tag="r")
                z_sb = work.tile([128, CH], ewdt, tag="z")
                nc.scalar.activation(r_sb[:], grz[:, 0:CH], Act.Sigmoid, bias=cb_r)
                nc.scalar.activation(z_sb[:], grz[:, CH:], Act.Sigmoid, bias=cb_z)

                # u = (g_hn + b_hn) * r
                u_sb = work.tile([128, CH], f32, tag="u")
                nc.vector.scalar_tensor_tensor(
                    u_sb[:], ghn[:, 0:CH], b_hn, r_sb[:], Alu.add, Alu.mult
                )
                # v = (g_in + b_in) + u
                v_sb = work.tile([128, CH], f32, tag="v")
                nc.vector.scalar_tensor_tensor(
                    v_sb[:], ghn[:, CH:], cb_in, u_sb[:], Alu.add, Alu.add
                )
                n_sb = work.tile([128, CH], ewdt, tag="n")
                nc.scalar.activation(n_sb[:], v_sb[:], Act.Tanh)

                # h' = n + z * (h - n)
                w_sb = work.tile([128, CH], ewdt, tag="w")
                nc.vector.tensor_sub(w_sb[:], h_prev[:, cs], n_sb[:])
                e_sb = work.tile([128, CH], ewdt, tag="e")
                nc.vector.tensor_mul(e_sb[:], z_sb[:], w_sb[:])
                nc.vector.tensor_add(h_new[:, cs], n_sb[:], e_sb[:])

                # y tiles for the 4 batch tiles in this chunk
                for jj in range(4):
                    j = c * 4 + jj
                    lhsT = h_new[:, j * 128 : (j + 1) * 128]
                    nc.tensor.matmul(py[:, j, :], lhsT, WoutT)

            # y = py + b_out -> staging
            nc.vector.tensor_tensor(
                Y[:, :, t, :],
                py[:, :, :],
                sbout[:, None, :].to_broadcast((128, NB, I)),
                Alu.add,
            )
            h_prev = h_new

        for j in range(NB):
            nc.sync.dma_start(
                out=dout[j * 128 : (j + 1) * 128, :, :], in_=Y[:, j, :, :]
            )

    return nc


def _host_prep(x, h, W_ih, W_hh, b_ih, b_hh, W_out, b_out):
    """Fold weights on the host (float64 for exactness), build per-core maps."""
    x = np.asarray(x, dtype=np.float32)
    h = np.asarray(h, dtype=np.float32)
    W_ih = np.asarray(W_ih, dtype=np.float64)
    W_hh = np.asarray(W_hh, dtype=np.float64)
    b_ih = np.asarray(b_ih, dtype=np.float64)
    b_hh = np.asarray(b_hh, dtype=np.float64)
    W_out = np.asarray(W_out, dtype=np.float64)
    b_out = np.asarray(b_out, dtype=np.float64)

    W_ih_eff = W_ih @ W_out  # [3H, H]
    b_ih_eff = W_ih @ b_out + b_ih  # [3H]

    f = np.float32
    WA = np.concatenate(
        [
            (W_hh[0:H] + W_ih_eff[0:H]).T,
            (W_hh[H : 2 * H] + W_ih_eff[H : 2 * H]).T,
            W_hh[2 * H : 3 * H].T,
            W_ih_eff[2 * H : 3 * H].T,
        ],
        axis=1,
    ).astype(f)  # [H, 4H]
    WA0 = np.concatenate([W_hh[0:H].T, W_hh[H : 2 * H].T], axis=1).astype(f)
    W0 = np.concatenate(
        [W_ih[0:H].T, W_ih[H : 2 * H].T, W_ih[2 * H : 3 * H].T], axis=1
    ).astype(f)  # [I, 3H]
    WoutT = np.ascontiguousarray(W_out.T).astype(f)  # [H, I]
    BIAS = np.stack(
        [
            b_hh[0:H] + b_ih_eff[0:H],
            b_hh[H : 2 * H] + b_ih_eff[H : 2 * H],
            b_hh[2 * H : 3 * H],
            b_ih_eff[2 * H : 3 * H],
            b_hh[0:H] + b_ih[0:H],
            b_hh[H : 2 * H] + b_ih[H : 2 * H],
            b_ih[2 * H : 3 * H],
            np.zeros(H),
        ],
        axis=1,
    ).astype(f)  # [H, 8]
    BOUT = np.ascontiguousarray(np.broadcast_to(b_out.astype(f), (128, I)))

    x0T = np.ascontiguousarray(x[:, SEQLEN, :].T)  # [I, B]
    h0T = np.ascontiguousarray(h[0].T)  # [H, B]

    in_maps = []
    for core in range(NCORES):
        cs = slice(core * BC, (core + 1) * BC)
        in_maps.append(
            {
                "x0t": np.ascontiguousarray(x0T[:, cs]),
                "h0t": np.ascontiguousarray(h0T[:, cs]),
                "wa": WA,
                "wa0": WA0,
                "w0": W0,
                "woutt": WoutT,
                "bias": BIAS,
                "bout": BOUT,
            }
        )
    return in_maps


def kernel(x, h, W_ih, W_hh, b_ih, b_hh, W_out, b_out):
    global LAST_RESULT
    from concourse.bass_utils import run_bass_kernel_spmd

    key = (MM_DT, EW_DT)
    if key not in _CACHE:
        nc = _build(MM_DT, EW_DT)
        # Bacc needs explicit finalize (wait-splitting, reg alloc);
        # run_bass_via_pjrt serializes the module as-is.
        nc.finalize()
        _CACHE[key] = nc
    nc = _CACHE[key]

    in_maps = _host_prep(x, h, W_ih, W_hh, b_ih, b_hh, W_out, b_out)
    res = run_bass_kernel_spmd(nc, in_maps, core_ids=list(range(NCORES)))
    LAST_RESULT = res
    out = np.concatenate([r["out"] for r in res.results], axis=0)
    return out


# revision 25
# speedup vs baseline: 6.7151x; 6.7151x over previous
"""Trainium2 Bass kernel for the autoregressive GRU decoder.

Reference computation (eval-mode Decoder):
  x0 = x[:, 30, :]                # only element of x ever used
  h0 = h[0]
  for t in 0..29:
      h = GRUCell(x_t, h)         # PyTorch gate layout [r, z, n]
      y_t = h @ W_out.T + b_out
      x_{t+1} = y_t               # linear feedback -> fold into weights
  out = stack(y_t)                # [B, 30, 32]

Because the feedback x_{t+1} = W_out @ h_t + b_out is linear, for t >= 1:
  gi_t = W_ih @ x_t + b_ih = (W_ih @ W_out) @ h_{t-1} + (W_ih @ b_out + b_ih)
so every step t >= 1 is a pure H->H recurrence.  Weights are folded on the
host; r/z gates use a single combined matrix (W_hh + W_ih_eff).

Device layout (per core, batch shard Bc = 2048):
  state h^T transposed [H=128 partitions, Bc free]; gate matmuls contract
  over H with weights stationary; gate biases ride in ACT bias /
  scalar_tensor_tensor scalar slots or as K=1 rank-1 matmuls into PSUM.
  y_t computed per 128-batch tile with h'^T stationary -> [batch, 32] PSUM,
  staged to SBUF and DMA'd out per step into a [STEPS, Bc*I] device layout
  (partition-major) that gives 2KB-contiguous DMA descriptors; the host
  un-permutes at the end.

Sharding: pure data parallel over batch, 8 cores x 2048, no collectives.
"""

import os

import numpy as np

B, T, I, H, SEQLEN = 16384, 60, 32, 128, 30
STEPS = T - SEQLEN  # 30
NCORES = 8
BC = B // NCORES  # 2048 batch rows per core
NB = BC // 128  # 16 batch tiles of 128
CH = 512  # matmul free-dim chunk (one PSUM bank)
NCH = BC // CH  # 4 chunks per step

# knobs (see _build)
MM_DT = os.environ.get("K_MM_DT", "f32r")  # f32 | f32r  (bf16 via K_EW_DT)
EW_DT = os.environ.get("K_EW_DT", "mixed")  # f32 | mixed | bf16
SFUSE = os.environ.get("K_SFUSE", "1") == "1"  # fused sigmoid + PE bias MMs
GPW = os.environ.get("K_GPW", "1") == "1"  # w = h - n on gpsimd
GPH = int(os.environ.get("K_GPH", "2"))  # how many of 4 h' chunks on gpsimd
YENG = os.environ.get("K_YENG", "act")  # act | dve: y psum->sbuf copy

LAST_RESULT = None  # BassKernelResults of the most recent run (for test.py)

_CACHE = {}


def _build(mm_dt_s, ew_dt_s, repeats=1):
    from contextlib import ExitStack

    import concourse.bacc as bacc
    import concourse.mybir as mybir
    import concourse.tile as tile

    f32 = mybir.dt.float32
    bf16 = mybir.dt.bfloat16
    f32r = mybir.dt.float32r
    Alu = mybir.AluOpType
    Act = mybir.ActivationFunctionType

    # dtypes:
    #  mmdt: matmul operand dtype (weights, state, x0).  f32r must be
    #  declared end-to-end; bf16 mode stores everything 2-byte.
    if ew_dt_s == "bf16":
        mmdt = bf16
    else:
        mmdt = {"f32": f32, "f32r": f32r}[mm_dt_s]
    hdt = mmdt if mmdt != f32 else f32  # state tiles feed matmuls
    # elementwise dtypes
    if ew_dt_s == "bf16":
        ndt = zdt = wdt_ = edt = bf16
    elif ew_dt_s == "mixed":
        ndt = f32  # n enters h' directly; keep fp32 to avoid random walk
        zdt = wdt_ = edt = bf16  # only scale the update delta
    else:
        ndt = zdt = wdt_ = edt = f32

    nc = bacc.Bacc()

    ddt = mmdt if mmdt == f32r else (bf16 if mmdt == bf16 else f32)
    dx = nc.dram_tensor("x0t", [I, BC], ddt, kind="ExternalInput")
    dh = nc.dram_tensor("h0t", [H, BC], ddt, kind="ExternalInput")
    dwa = nc.dram_tensor("wa", [H, 4 * H], ddt, kind="ExternalInput")
    dwa0 = nc.dram_tensor("wa0", [H, 2 * H], ddt, kind="ExternalInput")
    dw0 = nc.dram_tensor("w0", [I, 3 * H], ddt, kind="ExternalInput")
    dwout = nc.dram_tensor("woutt", [H, I], ddt, kind="ExternalInput")
    dbias = nc.dram_tensor("bias", [H, 8], f32, kind="ExternalInput")
    # row-layout biases for K=1 rank-1 PSUM bias matmuls, all on
    # partition 0 (matmul stationary base_partition must be 0/32/64):
    # [b_r | b_z | b0_r | b0_z | tile(b_out, 4)]
    dbrow = nc.dram_tensor("brow", [1, 5 * H], ddt, kind="ExternalInput")
    dout = nc.dram_tensor("out", [STEPS, BC * I], f32, kind="ExternalOutput")

    with ExitStack() as ctx:
        tc = ctx.enter_context(tile.TileContext(nc))
        const = ctx.enter_context(tc.tile_pool(name="const", bufs=1))
        state = ctx.enter_context(tc.tile_pool(name="state", bufs=2))
        work = ctx.enter_context(tc.tile_pool(name="work", bufs=3))
        psum = ctx.enter_context(tc.tile_pool(name="psum", bufs=1, space="PSUM"))

        def load_const(dram, shape, name):
            t = const.tile(shape, dram.dtype, tag=name)
            nc.sync.dma_start(out=t[:], in_=dram[:, :])
            return t

        swa = load_const(dwa, [H, 4 * H], "wa")
        swa0 = load_const(dwa0, [H, 2 * H], "wa0")
        sw0 = load_const(dw0, [I, 3 * H], "w0")
        swout = load_const(dwout, [H, I], "wout")
        sbias = load_const(dbias, [H, 8], "bias")
        sbrow = load_const(dbrow, [1, 5 * H], "brow")
        sx0 = load_const(dx, [I, BC], "x0")
        h0 = load_const(dh, [H, BC], "h0")

        sones = const.tile([1, CH], mmdt, tag="ones")
        nc.vector.memset(sones[:], 1.0)

        b_r = sbias[:, 0:1]
        b_z = sbias[:, 1:2]
        b_hn = sbias[:, 2:3]
        b_in = sbias[:, 3:4]
        b0_r = sbias[:, 4:5]
        b0_z = sbias[:, 5:6]
        b0_in = sbias[:, 6:7]

        A_r = swa[:, 0 * H : 1 * H]
        A_z = swa[:, 1 * H : 2 * H]
        A_hn = swa[:, 2 * H : 3 * H]
        A_in = swa[:, 3 * H : 4 * H]
        A0_r = swa0[:, 0 * H : 1 * H]
        A0_z = swa0[:, 1 * H : 2 * H]
        W0_r = sw0[:, 0 * H : 1 * H]
        W0_z = sw0[:, 1 * H : 2 * H]
        W0_n = sw0[:, 2 * H : 3 * H]
        WoutT = swout[:, :]
        brow_r = sbrow[0:1, 0 * H : 1 * H]
        brow_z = sbrow[0:1, 1 * H : 2 * H]
        brow0_r = sbrow[0:1, 2 * H : 3 * H]
        brow0_z = sbrow[0:1, 3 * H : 4 * H]
        brow_out = sbrow[0:1, 4 * H : 5 * H]  # tile(b_out, 4) -> [1, 128]
        ones128 = const.tile([1, H], mmdt, tag="ones128")
        nc.vector.memset(ones128[:], 1.0)
        browout_full = const.tile([1, NB * I], mmdt, tag="browout")
        # tile(b_out, NB): replicate [1, 4*32] four times via DMA-free copy
        for rr in range(4):
            nc.vector.tensor_copy(
                browout_full[:, rr * 4 * I : (rr + 1) * 4 * I], brow_out
            )

        def gru_step(t, h_prev, first):
            h_new = state.tile([H, BC], hdt, tag="h")
            py = psum.tile([128, NB, I], f32, tag="py", bufs=2)
            for c in range(NCH):
                cs = slice(c * CH, (c + 1) * CH)
                rhs = h_prev[:, cs]

                grz = psum.tile([128, 2 * CH], f32, tag="grz", bufs=2)
                ghn = psum.tile([128, 2 * CH], f32, tag="ghn", bufs=1)

                if first:
                    x_rhs = sx0[:, cs]
                    cb_r, cb_z, cb_in = b0_r, b0_z, b0_in
                    cbrow_r, cbrow_z = brow0_r, brow0_z
                    gr_mms = [(A0_r, rhs), (W0_r, x_rhs)]
                    gz_mms = [(A0_z, rhs), (W0_z, x_rhs)]
                    gin_mms = [(W0_n, x_rhs)]
                else:
                    cb_r, cb_z, cb_in = b_r, b_z, b_in
                    cbrow_r, cbrow_z = brow_r, brow_z
                    gr_mms = [(A_r, rhs)]
                    gz_mms = [(A_z, rhs)]
                    gin_mms = [(A_in, rhs)]
                # mixed mode needs r in fp32 but z in bf16 -> separate
                # sigmoid ops with ACT-side biases (no fused rz pass)
                sfuse = SFUSE and ew_dt_s != "mixed"
                if sfuse:
                    gr_mms.append((cbrow_r, sones[:, :]))
                    gz_mms.append((cbrow_z, sones[:, :]))

                for idx, (lh, rh) in enumerate(gr_mms):
                    nc.tensor.matmul(
                        grz[:, 0:CH], lh, rh,
                        start=(idx == 0), stop=(idx == len(gr_mms) - 1),
                    )
                for idx, (lh, rh) in enumerate(gz_mms):
                    nc.tensor.matmul(
                        grz[:, CH:], lh, rh,
                        start=(idx == 0), stop=(idx == len(gz_mms) - 1),
                    )
                nc.tensor.matmul(ghn[:, 0:CH], A_hn, rhs)
                for idx, (lh, rh) in enumerate(gin_mms):
                    nc.tensor.matmul(
                        ghn[:, CH:], lh, rh,
                        start=(idx == 0), stop=(idx == len(gin_mms) - 1),
                    )

                if sfuse:
                    rz_sb = work.tile([128, 2 * CH], zdt, tag="rz")
                    nc.scalar.activation(rz_sb[:], grz[:, :], Act.Sigmoid)
                    r_sb = rz_sb[:, 0:CH]
                    z_sb = rz_sb[:, CH:]
                else:
                    rdt = f32 if ew_dt_s == "mixed" else zdt
                    r_t = work.tile([128, CH], rdt, tag="r")
                    z_t = work.tile([128, CH], zdt, tag="z")
                    nc.scalar.activation(r_t[:], grz[:, 0:CH], Act.Sigmoid, bias=cb_r)
                    nc.scalar.activation(z_t[:], grz[:, CH:], Act.Sigmoid, bias=cb_z)
                    r_sb = r_t[:]
                    z_sb = z_t[:]

                # u = (g_hn + b_hn) * r ; v = (g_in + b_in) + u
                u_sb = work.tile([128, CH], f32, tag="u")
                nc.vector.scalar_tensor_tensor(
                    u_sb[:], ghn[:, 0:CH], b_hn, r_sb, Alu.add, Alu.mult
                )
                v_sb = work.tile([128, CH], f32, tag="v")
                nc.vector.scalar_tensor_tensor(
                    v_sb[:], ghn[:, CH:], cb_in, u_sb[:], Alu.add, Alu.add
                )
                n_sb = work.tile([128, CH], ndt, tag="n")
                nc.scalar.activation(n_sb[:], v_sb[:], Act.Tanh)

                # h' = n + z * (h - n)
                w_sb = work.tile([128, CH], wdt_, tag="w")
                weng = nc.gpsimd if GPW else nc.vector
                weng.tensor_tensor(w_sb[:], h_prev[:, cs], n_sb[:], Alu.subtract)
                e_sb = work.tile([128, CH], edt, tag="e")
                nc.vector.tensor_tensor(e_sb[:], z_sb, w_sb[:], Alu.mult)
                heng = nc.gpsimd if c < GPH else nc.vector
                heng.tensor_tensor(h_new[:, cs], n_sb[:], e_sb[:], Alu.add)

                # y tiles for the 4 batch tiles in this chunk
                for jj in range(4):
                    j = c * 4 + jj
                    lhsT = h_new[:, j * 128 : (j + 1) * 128]
                    nc.tensor.matmul(
                        py[:, j, :], lhsT, WoutT,
                        start=True, stop=False, skip_group_check=True,
                    )

            # y += b_out (rank-1), stage to SBUF, stream out
            nc.tensor.matmul(
                py[:, :, :], ones128, browout_full[:, :],
                start=False, stop=True, skip_group_check=True,
            )
            y_sb = work.tile([128, NB, I], f32, tag="y")
            if YENG == "act":
                nc.scalar.copy(y_sb[:, :, :], py[:, :, :])
            else:
                nc.vector.tensor_copy(y_sb[:, :, :], py[:, :, :])
            nc.sync.dma_start(
                out=dout[t, :].rearrange("(p a b) -> p a b", p=128, a=NB),
                in_=y_sb[:, :, :],
            )
            return h_new

        for _rep in range(repeats):
            h_prev = h0
            for t in range(STEPS):
                h_prev = gru_step(t, h_prev, t == 0)

    return nc


def _host_prep(x, h, W_ih, W_hh, b_ih, b_hh, W_out, b_out):
    """Fold weights on the host (float64 for exactness), build per-core maps."""
    x = np.asarray(x, dtype=np.float32)
    h = np.asarray(h, dtype=np.float32)
    W_ih = np.asarray(W_ih, dtype=np.float64)
    W_hh = np.asarray(W_hh, dtype=np.float64)
    b_ih = np.asarray(b_ih, dtype=np.float64)
    b_hh = np.asarray(b_hh, dtype=np.float64)
    W_out = np.asarray(W_out, dtype=np.float64)
    b_out = np.asarray(b_out, dtype=np.float64)

    W_ih_eff = W_ih @ W_out  # [3H, H]
    b_ih_eff = W_ih @ b_out + b_ih  # [3H]

    bf16_mode = EW_DT == "bf16"
    f = np.dtype("bfloat16") if False else np.float32  # device casts handled below

    def cvt(a):
        a = np.ascontiguousarray(a, dtype=np.float32)
        if bf16_mode:
            import ml_dtypes

            a = a.astype(ml_dtypes.bfloat16)
        return a

    WA = cvt(
        np.concatenate(
            [
                (W_hh[0:H] + W_ih_eff[0:H]).T,
                (W_hh[H : 2 * H] + W_ih_eff[H : 2 * H]).T,
                W_hh[2 * H : 3 * H].T,
                W_ih_eff[2 * H : 3 * H].T,
            ],
            axis=1,
        )
    )  # [H, 4H]
    WA0 = cvt(np.concatenate([W_hh[0:H].T, W_hh[H : 2 * H].T], axis=1))
    W0 = cvt(
        np.concatenate([W_ih[0:H].T, W_ih[H : 2 * H].T, W_ih[2 * H : 3 * H].T], axis=1)
    )  # [I, 3H]
    WoutT = cvt(W_out.T)  # [H, I]
    BIAS = np.ascontiguousarray(
        np.stack(
            [
                b_hh[0:H] + b_ih_eff[0:H],
                b_hh[H : 2 * H] + b_ih_eff[H : 2 * H],
                b_hh[2 * H : 3 * H],
                b_ih_eff[2 * H : 3 * H],
                b_hh[0:H] + b_ih[0:H],
                b_hh[H : 2 * H] + b_ih[H : 2 * H],
                b_ih[2 * H : 3 * H],
                np.zeros(H),
            ],
            axis=1,
        ),
        dtype=np.float32,
    )  # [H, 8]
    BROW = cvt(
        np.concatenate(
            [
                b_hh[0:H] + b_ih_eff[0:H],
                b_hh[H : 2 * H] + b_ih_eff[H : 2 * H],
                b_hh[0:H] + b_ih[0:H],
                b_hh[H : 2 * H] + b_ih[H : 2 * H],
                np.tile(b_out, 4),
            ]
        )[None, :]
    )  # [1, 5H]

    x0T = cvt(x[:, SEQLEN, :].T)  # [I, B]
    h0T = cvt(h[0].T)  # [H, B]

    in_maps = []
    for core in range(NCORES):
        cs = slice(core * BC, (core + 1) * BC)
        in_maps.append(
            {
                "x0t": np.ascontiguousarray(x0T[:, cs]),
                "h0t": np.ascontiguousarray(h0T[:, cs]),
                "wa": WA,
                "wa0": WA0,
                "w0": W0,
                "woutt": WoutT,
                "bias": BIAS,
                "brow": BROW,
            }
        )
    return in_maps


def _unshuffle(out_dev):
    """[STEPS, BC*I] device layout (row = p*NB + j) -> [BC, STEPS, I]."""
    x = out_dev.reshape(STEPS, 128, NB, I)
    return np.ascontiguousarray(x.transpose(2, 1, 0, 3).reshape(NB * 128, STEPS, I))


def _get_nc(repeats=1):
    key = (MM_DT, EW_DT, SFUSE, GPW, GPH, YENG, repeats)
    if key not in _CACHE:
        nc = _build(MM_DT, EW_DT, repeats)
        # Bacc needs explicit finalize (wait-splitting, reg alloc);
        # run_bass_via_pjrt serializes the module as-is.
        nc.finalize()
        _CACHE[key] = nc
    return _CACHE[key]


def run(in_maps, repeats=1):
    global LAST_RESULT
    from concourse.bass_utils import run_bass_kernel_spmd

    nc = _get_nc(repeats)
    res = run_bass_kernel_spmd(nc, in_maps, core_ids=list(range(NCORES)))
    LAST_RESULT = res
    return res


def gather(res):
    return np.concatenate([_unshuffle(r["out"]) for r in res.results], axis=0)


def kernel(x, h, W_ih, W_hh, b_ih, b_hh, W_out, b_out):
    in_maps = _host_prep(x, h, W_ih, W_hh, b_ih, b_hh, W_out, b_out)
    res = run(in_maps, repeats=1)
    return gather(res)
